# revision 8
# baseline (speedup 1.0000x reference)
"""Trainium2 Bass kernel for nn_MultiHeadAttention_76854144795234.

Multi-head attention with Shaw-style relative positional encodings
(clipped at +-512), faithful to the reference's head-mixing reshape
before the output projection.

Sharding: batch*head parallel over 8 cores. Core c owns batch b=c//2
and head group g=c%2 (heads 8g..8g+8 of that batch). Due to the
reference's out.reshape(H,B,S,HD).transpose(0,2,1,3) mixing, output
row (b, so) only consumes heads 4*((so//256)%4)+{0..3}, which for
so in [512g, 512g+512) is exactly head group g -- so each core
produces complete output rows (no cross-core reduction).

Rel-score term: QRx[q, m] = sum_d q[q,d]*relw[m,d] where relw is the
clipped window table (m = (k-q)+639, 1279 cols incl. duplicated flat
edges). QRx tiles round-trip through DRAM so a row-stride-1279
"diagonal" AP re-reads them shifted one per row, landing rel scores
in [q-partition, k-free] layout for the softmax. Flat regions beyond
the +-639 window are per-row constants folded into the exp() bias.

Value-side rel term: normalized attn (fp16, in DRAM) is re-read
through the DMA transpose engine with stride-1025 anti-diagonal APs
and contracted against an extended 2047-row value table; out-of-band
garbage is zeroed with affine_select on GPSIMD.
"""

import contextlib

import numpy as np

import concourse.bass as bass
from concourse import bacc
import concourse.mybir as mybir
import concourse.tile as tile
from concourse.bass_utils import run_bass_kernel_spmd
from concourse.masks import make_identity
from concourse.tile_rust import add_dep_helper

B, S, EMB, H, HD, MAXP = 4, 1024, 1024, 16, 64, 512
HPC = 8          # heads per core
JW = 512         # emb columns per head group
WIN = 1280       # rel window columns (1279 used + 1 dup pad)
PAD = 128        # leading pad elements of attn_flat
ENDPAD = 2048
F16 = mybir.dt.float16
F32 = mybir.dt.float32
AF = mybir.ActivationFunctionType
OP = mybir.AluOpType

N_CORES = 8


def _value_rel_pairs():
    """Static (a, bb, qlo, qhi, need_lo, need_hi) list for the value-side
    diagonal reads. j-tile a covers extended rows [128a, 128a+128);
    valid elements satisfy 1023 <= j+q <= 2046."""
    pairs = []
    for a in range(16):
        qlo_any = max(0, 896 - 128 * a)
        qhi_any = min(1024, 2047 - 128 * a)
        ql16 = (qlo_any // 16) * 16
        qh16 = min(1024, ((qhi_any + 15) // 16) * 16)
        for bb in range(2):
            lo = max(ql16, 512 * bb)
            hi = min(qh16, 512 * bb + 512)
            if lo >= hi:
                continue
            need_lo = (128 * a + lo) < 1023
            need_hi = (128 * a + 127 + hi - 1) > 2046
            pairs.append((a, bb, lo, hi, need_lo, need_hi))
    return pairs


def _emit(ctx, tc, nc, t_in, t_out):
    xq, xk, xv, wq, wk, wv, wo, bqv, relw, relv = t_in
    attn_flat, out_part = t_out

    singles = ctx.enter_context(tc.tile_pool(name="singles", bufs=1))
    bigx = ctx.enter_context(tc.tile_pool(name="bigx", bufs=4))
    ps_big = ctx.enter_context(tc.tile_pool(name="ps_big", bufs=2, space="PSUM"))
    ps_small = ctx.enter_context(tc.tile_pool(name="ps_small", bufs=2, space="PSUM"))
    ps_val = ctx.enter_context(tc.tile_pool(name="ps_val", bufs=2, space="PSUM"))
    qrx_sbp = ctx.enter_context(tc.tile_pool(name="qrx_sbp", bufs=3))
    band_p = ctx.enter_context(tc.tile_pool(name="band_p", bufs=3))
    exp_p = ctx.enter_context(tc.tile_pool(name="exp_p", bufs=3))
    attn_p = ctx.enter_context(tc.tile_pool(name="attn_p", bufs=3))
    attnt_p = ctx.enter_context(tc.tile_pool(name="attnt_p", bufs=3))
    ax_p = ctx.enter_context(tc.tile_pool(name="ax_p", bufs=4))
    tiny_p = ctx.enter_context(tc.tile_pool(name="tiny_p", bufs=12))
    outp_p = ctx.enter_context(tc.tile_pool(name="outp_p", bufs=2))
    qrx_dram = ctx.enter_context(tc.tile_pool(name="qrx_dram", bufs=6, space="DRAM"))

    # ---------------- constants / weights into SBUF ----------------
    # weight tiles, e-tile-major: w*_sb[p, 512*et + j] = W[128*et + p, j]
    wq_sb = singles.tile([128, 8 * JW], F16, tag="wq_sb")
    wk_sb = singles.tile([128, 8 * JW], F16, tag="wk_sb")
    wv_sb = singles.tile([128, 8 * JW], F16, tag="wv_sb")
    for w_sb, w in ((wq_sb, wq), (wk_sb, wk), (wv_sb, wv)):
        nc.sync.dma_start(out=w_sb.rearrange("p (et j) -> p et j", et=8),
                          in_=w.ap().rearrange("(et p) j -> p et j", p=128))

    relw_sb = singles.tile([128, WIN], F16, tag="relw_sb")
    nc.sync.dma_start(out=relw_sb, in_=relw.ap())
    # relv_sb[p, 64*a + d] = relv[128*a + p, d]
    relv_sb = singles.tile([128, 16 * HD], F16, tag="relv_sb")
    nc.sync.dma_start(out=relv_sb.rearrange("p (a d) -> p a d", a=16),
                      in_=relv.ap().rearrange("(a p) d -> p a d", p=128))

    ident = singles.tile([128, 128], F16, tag="ident")
    make_identity(nc, ident)

    # biases: b*_sb[p, jt] = b[128*jt + p]
    bq_sb = singles.tile([128, 4], F32, tag="bq_sb")
    bk_sb = singles.tile([128, 4], F32, tag="bk_sb")
    bv_sb = singles.tile([128, 4], F32, tag="bv_sb")
    for i, b_sb in enumerate((bq_sb, bk_sb, bv_sb)):
        nc.sync.dma_start(out=b_sb, in_=bass.AP(bqv, i * JW, [[1, 128], [128, 4]]))

    # x^T inputs, e-tile-major: x_sb[p, 1024*et + t]
    xq_sb = bigx.tile([128, 8 * S], F16, tag="bigx")
    xk_sb = bigx.tile([128, 8 * S], F16, tag="bigx")
    xv_sb = bigx.tile([128, 8 * S], F16, tag="bigx")
    for x_sb, x in ((xq_sb, xq), (xk_sb, xk), (xv_sb, xv)):
        nc.sync.dma_start(out=x_sb.rearrange("p (et t) -> p et t", et=8),
                          in_=x.ap().rearrange("(et p) t -> p et t", p=128))

    # long-lived products
    qT_sb = singles.tile([128, 4 * S], F16, tag="qT_sb")   # [j', t] j-tile-major
    kT_sb = singles.tile([128, 4 * S], F16, tag="kT_sb")
    v_sb = singles.tile([128, 8 * JW], F16, tag="v_sb")    # [t, j'] t-tile-major
    outT_sb = singles.tile([128, 4 * S], F16, tag="outT_sb")  # [j'', s] j-tile-major

    # ---------------- projections ----------------
    for (x_sb, w_sb, b_sb, dst) in ((xq_sb, wq_sb, bq_sb, qT_sb),
                                    (xk_sb, wk_sb, bk_sb, kT_sb)):
        for jt in range(4):
            for th in range(2):
                ps = ps_small.tile([128, 512], F32, tag="ps_small")
                for et in range(8):
                    nc.tensor.matmul(
                        ps,
                        lhsT=w_sb[:, 512 * et + 128 * jt: 512 * et + 128 * jt + 128],
                        rhs=x_sb[:, 1024 * et + 512 * th: 1024 * et + 512 * th + 512],
                        start=(et == 0), stop=(et == 7))
                nc.vector.tensor_scalar(
                    out=dst[:, 1024 * jt + 512 * th: 1024 * jt + 512 * th + 512],
                    in0=ps, scalar1=b_sb[:, jt: jt + 1], scalar2=None, op0=OP.add)

    for tt in range(8):
        ps = ps_small.tile([128, 512], F32, tag="ps_small")
        for et in range(8):
            nc.tensor.matmul(
                ps,
                lhsT=xv_sb[:, 1024 * et + 128 * tt: 1024 * et + 128 * tt + 128],
                rhs=wv_sb[:, 512 * et: 512 * et + 512],
                start=(et == 0), stop=(et == 7))
        # bv is added at the value-stage output (rows of attn sum to 1).
        nc.vector.tensor_copy(v_sb[:, 512 * tt: 512 * tt + 512], ps)

    # ---------------- attention ----------------
    vr_pairs = _value_rel_pairs()
    last_rel_idx = {bb: max(i for i, p in enumerate(vr_pairs) if p[1] == bb)
                    for bb in (0, 1)}

    for head in range(HPC):
        base = 64 * (head % 2)
        jt = head // 2

        def qcol(t):
            return 1024 * jt + 128 * t

        attn_writes = []
        qrx_tiles = []

        # --- QRx window product, staged through DRAM ---
        for t in range(8):
            qrx_sb = qrx_sbp.tile([128, WIN], F16, tag="qrx_sb")
            for ci, (c0, cw) in enumerate(((0, 512), (512, 512), (1024, 256))):
                ps = ps_small.tile([128, 512], F32, tag="ps_small")
                nc.tensor.matmul(
                    ps[:, :cw],
                    lhsT=qT_sb[base: base + 64, qcol(t): qcol(t) + 128],
                    rhs=relw_sb[base: base + 64, c0: c0 + cw],
                    start=True, stop=True)
                if ci == 1:
                    nc.scalar.copy(qrx_sb[:, c0: c0 + cw], ps[:, :cw])
                else:
                    nc.vector.tensor_copy(qrx_sb[:, c0: c0 + cw], ps[:, :cw])
            qrx_dt = qrx_dram.tile([128, WIN], F16, tag="qrx_dram")
            nc.sync.dma_start(out=qrx_dt, in_=qrx_sb)
            qrx_tiles.append((qrx_dt, qrx_sb))

        # --- scores + softmax per q-tile ---
        for t in range(8):
            qrx_dt, qrx_sb = qrx_tiles[t]
            k_lo = max(0, 128 * (t - 4))
            k_hi = min(S, 128 * (t + 5))
            bw = k_hi - k_lo

            band = band_p.tile([128, 1024], F16, tag="band")
            nc.sync.dma_start(
                out=band[:, :bw],
                in_=bass.AP(qrx_dt.tensor,
                            qrx_dt.offset + (k_lo - 128 * t) + 639,
                            [[WIN - 1, 128], [1, bw]]))

            ps = ps_big.tile([128, 1024], F32, tag="ps_big")
            for n in range(2):
                nc.tensor.matmul(
                    ps[:, 512 * n: 512 * n + 512],
                    lhsT=qT_sb[base: base + 64, qcol(t): qcol(t) + 128],
                    rhs=kT_sb[base: base + 64,
                              1024 * jt + 512 * n: 1024 * jt + 512 * n + 512],
                    start=True, stop=False, skip_group_check=True)
            for (p0, p1) in ((k_lo, min(512, k_hi)), (max(512, k_lo), k_hi)):
                if p0 >= p1:
                    continue
                nc.tensor.matmul(
                    ps[:, p0:p1],
                    lhsT=ident,
                    rhs=band[:, p0 - k_lo: p1 - k_lo],
                    start=False, stop=True, skip_group_check=True)

            exp_sb = exp_p.tile([128, 1024], F32, tag="exp")
            segs = []
            if k_lo > 0:
                segs.append((0, k_lo, 0))            # left flat: window col 0
            segs.append((k_lo, k_hi, None))
            if k_hi < S:
                segs.append((k_hi, S, WIN - 2))      # right flat: window col 1278
            accs = []
            for (s0, s1, flat) in segs:
                acc = tiny_p.tile([128, 1], F32, tag="acc")
                if flat is None:
                    bias = 0.0
                else:
                    bias = tiny_p.tile([128, 1], F32, tag="bias")
                    nc.vector.tensor_scalar(
                        out=bias, in0=qrx_sb[:, flat: flat + 1],
                        scalar1=0.125, scalar2=None, op0=OP.mult)
                nc.scalar.activation(
                    out=exp_sb[:, s0:s1], in_=ps[:, s0:s1], func=AF.Exp,
                    bias=bias, scale=0.125, accum_out=acc)
                accs.append(acc)

            if len(accs) == 3:
                z = tiny_p.tile([128, 1], F32, tag="z")
                nc.vector.scalar_tensor_tensor(
                    out=z, in0=accs[0], scalar=accs[1], in1=accs[2],
                    op0=OP.add, op1=OP.add)
            elif len(accs) == 2:
                z = tiny_p.tile([128, 1], F32, tag="z")
                nc.vector.tensor_add(z, accs[0], accs[1])
            else:
                z = accs[0]
            rz = tiny_p.tile([128, 1], F32, tag="rz")
            nc.vector.reciprocal(rz, z)

            attn_t = attn_p.tile([128, 1024], F16, tag="attn")
            nc.vector.tensor_scalar(
                out=attn_t, in0=exp_sb, scalar1=rz, scalar2=None, op0=OP.mult)
            wi = nc.sync.dma_start(
                out=bass.AP(attn_flat, PAD + head * S * S + 1024 * 128 * t,
                            [[1024, 128], [1, 1024]]),
                in_=attn_t)
            attn_writes.append(wi)

        # --- value stage: out^T[j', q] rows for this head ---
        pv0 = ps_val.tile([64, 512], F32, tag="ps_val")
        pv1 = ps_val.tile([64, 512], F32, tag="ps_val")
        pv = (pv0, pv1)

        for c in range(8):
            at = attnt_p.tile([128, 1024], F16, tag="attnt")
            ri = nc.sync.dma_start_transpose(
                out=at,
                in_=bass.AP(attn_flat, PAD + head * S * S + 128 * c,
                            [[1024, 1024], [1, 128]]))
            for w in attn_writes:
                add_dep_helper(ri.ins, w.ins, True, "attn write -> T read")
            for bb in range(2):
                nc.tensor.matmul(
                    pv[bb],
                    lhsT=v_sb[:, 512 * c + 64 * head: 512 * c + 64 * head + 64],
                    rhs=at[:, 512 * bb: 512 * bb + 512],
                    start=(c == 0), stop=False, skip_group_check=True)

        for pi, (a, bb, lo, hi, need_lo, need_hi) in enumerate(vr_pairs):
            n = hi - lo
            ax = ax_p.tile([128, 512], F16, tag="ax")
            ri = nc.sync.dma_start_transpose(
                out=ax[:, :n],
                in_=bass.AP(attn_flat,
                            PAD + head * S * S + 1025 * lo + 128 * a - 1023,
                            [[1025, n], [1, 128]]))
            for w in attn_writes:
                add_dep_helper(ri.ins, w.ins, True, "attn write -> diag read")
            if need_lo:
                nc.gpsimd.affine_select(
                    out=ax[:, :n], in_=ax[:, :n], pattern=[[1, n]],
                    compare_op=OP.is_ge, fill=0.0,
                    base=128 * a + lo - 1023, channel_multiplier=1)
            if need_hi:
                nc.gpsimd.affine_select(
                    out=ax[:, :n], in_=ax[:, :n], pattern=[[-1, n]],
                    compare_op=OP.is_ge, fill=0.0,
                    base=2046 - 128 * a - lo, channel_multiplier=-1)
            nc.tensor.matmul(
                pv[bb][:, lo - 512 * bb: lo - 512 * bb + n],
                lhsT=relv_sb[:, 64 * a: 64 * a + 64],
                rhs=ax[:, :n],
                start=False, stop=(pi == last_rel_idx[bb]),
                skip_group_check=True)

        for bb in range(2):
            nc.vector.tensor_scalar(
                out=outT_sb[base: base + 64,
                            1024 * jt + 512 * bb: 1024 * jt + 512 * bb + 512],
                in0=pv[bb],
                scalar1=bv_sb[base: base + 64, jt: jt + 1],
                scalar2=None, op0=OP.add)

    # ---------------- output projection (permuted) ----------------
    wo_sb = bigx.tile([128, 8 * S], F16, tag="bigx")
    nc.sync.dma_start(out=wo_sb.rearrange("p (jt e) -> p jt e", jt=8),
                      in_=wo.ap().rearrange("(jt p) e -> p jt e", p=128))

    for t_o in range(4):
        gl = t_o // 2
        par = t_o % 2
        ps = ps_big.tile([128, 1024], F32, tag="ps_big")
        nmm = 0
        for al in range(4):
            for b2 in range(2):
                f0 = 1024 * (2 * gl + b2) + 512 * par
                lhs = outT_sb[:, f0: f0 + 512].rearrange(
                    "p (po a) -> p po a", a=4)[:, :, al]
                for n in range(2):
                    nc.tensor.matmul(
                        ps[:, 512 * n: 512 * n + 512],
                        lhsT=lhs,
                        rhs=wo_sb[:, 1024 * (2 * al + b2) + 512 * n:
                                  1024 * (2 * al + b2) + 512 * n + 512],
                        start=(nmm == 0), stop=(al == 3 and b2 == 1),
                        skip_group_check=True)
                nmm += 1
        o_sb = outp_p.tile([128, 1024], F32, tag="outp")
        nc.vector.tensor_copy(o_sb, ps)
        nc.sync.dma_start(out=out_part.ap()[128 * t_o: 128 * t_o + 128, :], in_=o_sb)


_CACHE = {}


def _get_nc():
    if "nc" not in _CACHE:
        nc = bacc.Bacc("TRN2", target_bir_lowering=False, debug=False,
                       num_devices=N_CORES)
        xq = nc.dram_tensor("xq", [EMB, S], F16, kind="ExternalInput")
        xk = nc.dram_tensor("xk", [EMB, S], F16, kind="ExternalInput")
        xv = nc.dram_tensor("xv", [EMB, S], F16, kind="ExternalInput")
        wq = nc.dram_tensor("wq", [EMB, JW], F16, kind="ExternalInput")
        wk = nc.dram_tensor("wk", [EMB, JW], F16, kind="ExternalInput")
        wv = nc.dram_tensor("wv", [EMB, JW], F16, kind="ExternalInput")
        wo = nc.dram_tensor("wo", [EMB, EMB], F16, kind="ExternalInput")
        bqv = nc.dram_tensor("bqv", [3, JW], F32, kind="ExternalInput")
        relw = nc.dram_tensor("relw", [128, WIN], F16, kind="ExternalInput")
        relv = nc.dram_tensor("relv", [2048, HD], F16, kind="ExternalInput")
        attn_flat = nc.dram_tensor("attn_flat", [PAD + HPC * S * S + ENDPAD], F16,
                                   kind="ExternalOutput")
        out_part = nc.dram_tensor("out_part", [JW, EMB], F32, kind="ExternalOutput")

        with contextlib.ExitStack() as ctx:
            tc = ctx.enter_context(tile.TileContext(nc))
            _emit(ctx, tc, nc,
                  (xq, xk, xv, wq, wk, wv, wo, bqv, relw, relv),
                  (attn_flat, out_part))
        nc.compile()
        _CACHE["nc"] = nc
    return _CACHE["nc"]


def _host_tables(rel_k, rel_v):
    m = np.arange(WIN)
    relw_np = rel_k[np.clip(m - 639, -MAXP, MAXP) + MAXP, :].T  # [64, 1280]
    relw_np = np.concatenate([relw_np, relw_np], axis=0)        # dup for base-64
    j = np.arange(2048)
    relv_np = rel_v[np.clip(j - 1023, -MAXP, MAXP) + MAXP, :]   # [2048, 64]
    return (np.ascontiguousarray(relw_np, dtype=np.float16),
            np.ascontiguousarray(relv_np, dtype=np.float16))


def kernel(query, key, value, Wq, bq, Wk, bk, Wv, bv, Wo, bo, rel_k, rel_v):
    query = np.asarray(query, np.float32)
    key = np.asarray(key, np.float32)
    value = np.asarray(value, np.float32)
    Wq, Wk, Wv, Wo = (np.asarray(a, np.float32) for a in (Wq, Wk, Wv, Wo))
    bq, bk, bv, bo = (np.asarray(a, np.float32) for a in (bq, bk, bv, bo))
    rel_k = np.asarray(rel_k, np.float32)
    rel_v = np.asarray(rel_v, np.float32)

    nc = _get_nc()
    relw_np, relv_np = _host_tables(rel_k, rel_v)
    wo_np = np.ascontiguousarray(Wo, dtype=np.float16)

    in_maps = []
    for c in range(N_CORES):
        b, g = c // 2, c % 2
        cols = slice(JW * g, JW * g + JW)
        in_maps.append({
            "xq": np.ascontiguousarray(query[b].T, dtype=np.float16),
            "xk": np.ascontiguousarray(key[b].T, dtype=np.float16),
            "xv": np.ascontiguousarray(value[b].T, dtype=np.float16),
            "wq": np.ascontiguousarray(Wq[:, cols], dtype=np.float16),
            "wk": np.ascontiguousarray(Wk[:, cols], dtype=np.float16),
            "wv": np.ascontiguousarray(Wv[:, cols], dtype=np.float16),
            "wo": wo_np,
            "bqv": np.ascontiguousarray(
                np.stack([bq[cols], bk[cols], bv[cols]]), dtype=np.float32),
            "relw": relw_np,
            "relv": relv_np,
        })

    import os
    trace = bool(os.environ.get("KERNEL_TRACE"))
    res = run_bass_kernel_spmd(nc, in_maps, list(range(N_CORES)), trace=trace)
    kernel.last_result = res
    results = res.results

    attn = np.empty((B * H, S, S), np.float32)
    out = np.empty((B, S, EMB), np.float32)
    for c in range(N_CORES):
        b, g = c // 2, c % 2
        af = results[c]["attn_flat"][PAD: PAD + HPC * S * S].reshape(HPC, S, S)
        attn[b * H + 8 * g: b * H + 8 * g + HPC] = af.astype(np.float32)
        out[b, JW * g: JW * g + JW, :] = results[c]["out_part"] + bo[None, :]
    return out, attn
